# revision 1
# baseline (speedup 1.0000x reference)
"""Haar DWT (2x2, stride 2) on Trainium2 via Bass/Tile.

Full input  x : (4, 64, 512, 512) fp32
Full output   : (4, 256, 256, 256) fp32, channel = c*4 + band, bands [ll,lh,hl,hh]

Sharding: purely data-parallel. The 256 (batch, channel) images of 512x512 are
split 32-per-core across 8 NeuronCores; each image is independent.

Per-core program (SPMD, same NEFF on all 8 cores):
  per image m (32 total):
    - one contiguous 1 MiB DMA HBM->SBUF into t[128, 2048]
      (partition p holds input rows 4p..4p+3; free dim = [rp(2), eo(2), w(512)])
    - ScalarE: th = 0.5 * t                      (prescale; 0.5*H(x) == H(0.5x))
    - DVE:     vs = th[even rows] + th[odd rows] (vertical sum,  [128, 2x512])
    - GpSimd:  vd = th[even rows] - th[odd rows] (vertical diff, [128, 2x512])
    - DVE:     ll = vs[::2]+vs[1::2], lh = vs[::2]-vs[1::2]
               hl = vd[::2]+vd[1::2], hh = vd[::2]-vd[1::2]   (horizontal stage)
      written into ob[128, 2048] laid out as [k(4), rp(2), w(256)]
    - one 1 MiB DMA SBUF->HBM to out[m] (4, 256, 256), 2 KiB contiguous chunks
Work is spread across ScalarE/DVE/GpSimd so every engine stays under the
per-core HBM roofline (~180 us for 32 MiB in + 32 MiB out at ~358 GB/s).
"""

import numpy as np

import concourse.bacc as bacc
import concourse.mybir as mybir
import concourse.tile as tile
from concourse.bass_utils import run_bass_kernel_spmd

N_CORES = 8
B, C, H, W = 4, 64, 512, 512
IMGS = (B * C) // N_CORES  # 32 images per core
PART = 128
FREE = (H * W) // PART  # 2048 fp32 per partition per image
HO, WO = H // 2, W // 2

_cache = {}


def _build():
    nc = bacc.Bacc(
        "TRN2", target_bir_lowering=False, debug=False, enable_asserts=False
    )
    f32 = mybir.dt.float32
    x = nc.dram_tensor("x", [IMGS, PART, FREE], f32, kind="ExternalInput").ap()
    out = nc.dram_tensor("out", [IMGS, 4, HO, WO], f32, kind="ExternalOutput").ap()

    with tile.TileContext(nc) as tc:
        with (
            tc.tile_pool(name="tin", bufs=3) as tin,
            tc.tile_pool(name="tsc", bufs=2) as tsc,
            tc.tile_pool(name="tv", bufs=2) as tv,
            tc.tile_pool(name="tob", bufs=3) as tob,
        ):
            for m in range(IMGS):
                t = tin.tile([PART, FREE], f32)
                nc.sync.dma_start(t[:], x[m])

                th = tsc.tile([PART, FREE], f32)
                nc.scalar.mul(th[:], t[:], 0.5)

                thv = th[:].rearrange("p (rp eo w) -> p rp eo w", rp=2, eo=2)
                e, o = thv[:, :, 0, :], thv[:, :, 1, :]

                vs = tv.tile([PART, FREE // 2], f32, tag="vs")
                vd = tv.tile([PART, FREE // 2], f32, tag="vd")
                nc.vector.tensor_add(
                    vs[:].rearrange("p (rp w) -> p rp w", rp=2), e, o
                )
                nc.gpsimd.tensor_sub(
                    vd[:].rearrange("p (rp w) -> p rp w", rp=2), e, o
                )

                ob = tob.tile([PART, FREE], f32)
                obv = ob[:].rearrange("p (k rp w) -> p k rp w", k=4, rp=2)
                vs2 = vs[:].rearrange("p (rp w two) -> p rp w two", rp=2, two=2)
                vd2 = vd[:].rearrange("p (rp w two) -> p rp w two", rp=2, two=2)
                s0, s1 = vs2[:, :, :, 0], vs2[:, :, :, 1]
                d0, d1 = vd2[:, :, :, 0], vd2[:, :, :, 1]
                nc.vector.tensor_add(obv[:, 0], s0, s1)  # ll
                nc.vector.tensor_sub(obv[:, 1], s0, s1)  # lh
                nc.vector.tensor_add(obv[:, 2], d0, d1)  # hl
                nc.vector.tensor_sub(obv[:, 3], d0, d1)  # hh

                dst = out[m].rearrange("k (p rp) w -> p k rp w", p=PART)
                nc.scalar.dma_start(dst, obv)

    nc.compile()
    return nc


def _get_nc():
    if "nc" not in _cache:
        _cache["nc"] = _build()
    return _cache["nc"]


def run(x, trace=False):
    """Run on 8 cores; returns (full_output, BassKernelResults)."""
    x = np.ascontiguousarray(np.asarray(x, dtype=np.float32))
    assert x.shape == (B, C, H, W)
    nc = _get_nc()
    shards = x.reshape(N_CORES, IMGS, PART, FREE)
    in_maps = [{"x": shards[c]} for c in range(N_CORES)]
    res = run_bass_kernel_spmd(
        nc, in_maps, core_ids=list(range(N_CORES)), trace=trace
    )
    outs = np.stack([res.results[c]["out"] for c in range(N_CORES)])
    full = outs.reshape(B, C, 4, HO, WO).reshape(B, 4 * C, HO, WO)
    return full, res


def kernel(x):
    full, _ = run(x, trace=False)
    return full
